# revision 33
# baseline (speedup 1.0000x reference)
"""Trainium2 Bass kernel for nn_Attention (B=8, N=1024, C=768, 12 heads).

Sharding: pure data-parallel over batch — 8 cores, one batch element per
core, full weights replicated to every core. No collectives.

Per-core dataflow (tokens N=1024, channels C=768, heads 12 x 64), all
tensor-engine matmuls in bf16 (f32 PSUM accumulation):
  stage 0: PE warm-up burst (HAM clock gate) while the input DMAs land;
           bias broadcast via gpsimd partition_broadcast.
  stage 1: v = xT_tile.T @ w_v, token-major, stored as v_aug per-head
           tiles [128, 128]: cols 0:64 = v, col 64 = ones, rest unused.
           Pair-0 q/k projection groups interleaved into the tail.
  stage 2 (attention, one head at a time, head pair hp shares qkT tiles
           in partitions 0-63 / 64-127):
           S^T[tk,tq] = kT_h.T @ qT_h            (K=64)
           attn_exp = exp(scale * S^T)           (ACT, PSUM -> bf16 SBUF)
           O^T_unnorm[65,tq] = v_aug.T @ attn_exp — the ones column makes
           row 64 the softmax denominator for free
           normalize: DVE reciprocal of the sums row, gpsimd
           partition_broadcast to 64 partitions, DVE multiply -> oT bf16.
           The q/k projection of the NEXT head pair (matmuls + DVE
           PSUM->SBUF drains) is interleaved into the mk loop so the PE
           never starves while ACT computes exp; the next head's first
           score tile is prefetched at mk==7.
  stage 3: final[tq,:] = O^T_norm.T @ w_proj; bias added during the
           PSUM->SBUF drain (DVE); per-half DMA out.

Inputs x/w_qkv/w_proj are pre-converted to bf16 on the host (x also
pre-transposed) — halves DMA and runs the PE at 1 cycle/row.
"""

import os
import sys

import numpy as np

for _p in ("/opt/trn_rl_repo",):
    if os.path.isdir(_p) and _p not in sys.path:
        sys.path.append(_p)

import ml_dtypes

import concourse.bacc as bacc
import concourse.mybir as mybir
import concourse.tile as tile
from concourse.bass_utils import run_bass_kernel_spmd

F32 = mybir.dt.float32
BF16 = mybir.dt.bfloat16
EXP = mybir.ActivationFunctionType.Exp

P = 128
B, N, C = 8, 1024, 768
NH, HD = 12, 64
C3 = 3 * C
KC = C // P          # 6 contraction tiles over channels
NT = N // P          # 8 token tiles of 128
NT2 = N // 512       # 2 token slices of 512
VA = HD + 1          # 65: head dim + ones column
VP = VA              # per-head stride inside v_aug tiles (dense, no pad)
SCALE = float(HD) ** -0.5

_CACHE = {}


def _emit(nc, tc):
    xT_d = nc.dram_tensor("xT", [C, N], BF16, kind="ExternalInput")
    wqkv_d = nc.dram_tensor("w_qkv", [C, C3], BF16, kind="ExternalInput")
    wproj_d = nc.dram_tensor("w_proj", [C, C], BF16, kind="ExternalInput")
    bproj_d = nc.dram_tensor("b_proj", [1, C], F32, kind="ExternalInput")
    out_d = nc.dram_tensor("out", [N, C], F32, kind="ExternalOutput")

    mm = nc.tensor.matmul

    from contextlib import ExitStack

    with ExitStack() as es:
        const = es.enter_context(tc.tile_pool(name="const", bufs=1))
        big = es.enter_context(tc.tile_pool(name="big", bufs=1))

        bproj_sb = const.tile([1, C], F32, tag="bproj", name="bproj")
        nc.sync.dma_start(bproj_sb[:], bproj_d.ap())
        bias_bc = const.tile([P, C], F32, tag="bias_bc", name="bias_bc")

        xT = [big.tile([P, N], BF16, tag=f"xT{k}", name=f"xT{k}")
              for k in range(KC)]
        wqv = [big.tile([P, C], BF16, tag=f"wqv{k}", name=f"wqv{k}")
               for k in range(KC)]
        wqk = [big.tile([P, 2 * C], BF16, tag=f"wqk{k}", name=f"wqk{k}")
               for k in range(KC)]
        wpa = big.tile([P, KC * C], BF16, tag="wpa", name="wpa")
        wproj_sb = [wpa[:, k * C:(k + 1) * C] for k in range(KC)]
        qkT = [big.tile([P, N], BF16, tag=f"qk{m}", name=f"qk{m}")
               for m in range(2 * KC)]
        vaug = [big.tile([P, NH * VP], BF16, tag=f"va{t}", name=f"va{t}")
                for t in range(NT)]
        oT = [big.tile([P, N], BF16, tag=f"oT{i}", name=f"oT{i}")
              for i in range(KC)]

        # ---------------- stage 0 + 1: warmup, DMA in, v ----------------
        with tc.tile_pool(name="warmp", bufs=1) as warmp, \
             tc.tile_pool(name="ps_w", bufs=1, space="PSUM") as ps_w, \
             tc.tile_pool(name="ps_v", bufs=2, space="PSUM") as ps_v, \
             tc.tile_pool(name="ps_q0", bufs=2, space="PSUM") as ps_q0:
            # Warm the PE clock (HAM) with dummy matmuls while DMAs land.
            warm_sb = warmp.tile([P, 512], BF16, tag="warm", name="warm")
            nc.vector.memset(warm_sb[:], 0.0)
            warm_ps = ps_w.tile([P, 512], F32, tag="warmps", name="warmps")
            for _ in range(16):
                mm(warm_ps[:], warm_sb[:, 0:P], warm_sb[:],
                   start=True, stop=True)

            # Each issuing engine owns a hardware DMA queue: transfers are
            # FIFO within a queue but share HBM bandwidth across queues
            # (the gpsimd-issued queue gets the biggest share). Enqueue in
            # need-order everywhere: v-projection inputs (xT + v-columns,
            # k-ascending) first on every queue, then q/k weight columns,
            # then w_proj.
            urgent = []
            for k in range(KC):
                urgent.append((xT[k][:], xT_d.ap()[k * P:(k + 1) * P, :]))
                urgent.append((wqv[k][:],
                               wqkv_d.ap()[k * P:(k + 1) * P, 2 * C:]))
            # weighted split: gpsimd queue drains ~3x faster
            engs = [nc.gpsimd, nc.gpsimd, nc.sync, nc.gpsimd, nc.gpsimd,
                    nc.scalar] * 2
            for (dst, src), eng in zip(urgent, engs):
                eng.dma_start(dst, src)
            for k in range(KC):
                eng = (nc.gpsimd, nc.gpsimd, nc.scalar,
                       nc.gpsimd, nc.gpsimd, nc.sync)[k]
                eng.dma_start(wqk[k][:],
                              wqkv_d.ap()[k * P:(k + 1) * P, 0:2 * C])
            wp_src = wproj_d.ap().rearrange("(k p) j -> p k j", p=P)
            nc.gpsimd.dma_start(wpa[:].rearrange("p (k j) -> p k j", j=C),
                                wp_src)
            nc.gpsimd.partition_broadcast(bias_bc[:], bproj_sb[:])

            def qk_group(m, n2, drains):
                """One q/k projection group: 6 matmuls + DVE drain."""
                ps = ps_q0.tile([P, 512], F32, tag="psq", name="psq")
                for k in range(KC):
                    mm(ps[:], wqk[k][:, m * P:(m + 1) * P],
                       xT[k][:, n2 * 512:(n2 + 1) * 512],
                       start=(k == 0), stop=(k == KC - 1))
                drains.append(
                    lambda m=m, n2=n2, ps=ps: nc.vector.tensor_copy(
                        qkT[m][:, n2 * 512:(n2 + 1) * 512], ps[:]))

            # v: token-major [tokens 128, feat], scattered into v_aug tiles
            # (per-head stride VP=65: cols 0:64 data, col 64 ones).
            for t in range(NT):
                nc.gpsimd.memset(
                    vaug[t][:].rearrange("p (h m) -> p h m", m=VP)
                    [:, :, HD:HD + 1], 1.0)
                psa = ps_v.tile([P, 512], F32, tag="psva", name="psva")
                psb = ps_v.tile([P, 256], F32, tag="psvb", name="psvb")
                for k in range(KC):
                    mm(psa[:], xT[k][:, t * P:(t + 1) * P],
                       wqv[k][:, 0:512],
                       start=(k == 0), stop=(k == KC - 1))
                    mm(psb[:], xT[k][:, t * P:(t + 1) * P],
                       wqv[k][:, 512:768],
                       start=(k == 0), stop=(k == KC - 1))
                dst = vaug[t][:].rearrange("p (h m) -> p h m", m=VP)
                srca = psa[:].rearrange("p (h m) -> p h m", m=HD)
                srcb = psb[:].rearrange("p (h m) -> p h m", m=HD)
                nc.vector.tensor_copy(dst[:, 0:8, 0:HD], srca)
                nc.vector.tensor_copy(dst[:, 8:12, 0:HD], srcb)
            # pair-0 q/k projection (q/k weight columns have landed by now)
            d0 = []
            for m in (0, KC):
                for n2 in range(NT2):
                    qk_group(m, n2, d0)
                    d0.pop(0)()

        # ---------------- stage 2: attention ----------------
        with tc.tile_pool(name="attn", bufs=6) as attn_pool, \
             tc.tile_pool(name="small", bufs=4) as small, \
             tc.tile_pool(name="outp", bufs=3) as out_pool, \
             tc.tile_pool(name="ps_s", bufs=2, space="PSUM") as ps_s, \
             tc.tile_pool(name="ps_pv", bufs=3, space="PSUM") as ps_pv, \
             tc.tile_pool(name="ps_qk", bufs=1, space="PSUM") as ps_qk:

            def s_tile(h, mk):
                """Score tile S^T[tk block mk, all tq] for head h."""
                qt, kt = qkT[h // 2], qkT[KC + h // 2]
                hr = slice((h % 2) * HD, (h % 2) * HD + HD)
                s = ps_s.tile([P, N], F32, tag="s", name="s")
                tkc = slice(mk * P, (mk + 1) * P)
                for n2 in range(NT2):
                    mm(s[:, n2 * 512:(n2 + 1) * 512], kt[hr, tkc],
                       qt[hr, n2 * 512:(n2 + 1) * 512],
                       start=True, stop=True)
                return s

            # pending q/k projection work for the next pair, interleaved
            # one matmul at a time into the attention inner loop
            pend = []    # flat list of (m, n2, k) matmuls still to emit
            drains = []  # deferred DVE PSUM->SBUF drains
            cur_ps = [None]
            psa0 = [None]  # proj t=0 psa partials, filled during pair 5

            def emit_qk(n_items):
                for _ in range(n_items):
                    if not pend:
                        return
                    m, n2, k = pend.pop(0)
                    if k == 0:
                        while drains:
                            drains.pop(0)()
                        cur_ps[0] = ps_qk.tile([P, 512], F32, tag="psq2",
                                               name="psq2")
                    ps = cur_ps[0]
                    mm(ps[:], wqk[k][:, m * P:(m + 1) * P],
                       xT[k][:, n2 * 512:(n2 + 1) * 512],
                       start=(k == 0), stop=(k == KC - 1))
                    if k == KC - 1:
                        drains.append(
                            lambda m=m, n2=n2, ps=ps: nc.vector.tensor_copy(
                                qkT[m][:, n2 * 512:(n2 + 1) * 512], ps[:]))

            s_cur = s_tile(0, 0)
            for hp in range(KC):
                if hp + 1 < KC:
                    pend = [(m, n2, k)
                            for m in (hp + 1, KC + hp + 1)
                            for n2 in range(NT2)
                            for k in range(KC)]
                for half in range(2):
                    h = 2 * hp + half
                    hr = slice(half * HD, (half + 1) * HD)
                    pv = [ps_pv.tile([VA, 512], F32, tag="pv", name="pv")
                          for _ in range(NT2)]
                    for mk in range(NT):
                        et = attn_pool.tile([P, N], BF16, tag="e", name="e")
                        nc.scalar.activation(et[:], s_cur[:], EXP,
                                             scale=SCALE)
                        if half == 1 and mk == 5:
                            # finish the next pair's q/k projection well
                            # before the cross-pair score prefetch reads it
                            emit_qk(len(pend))
                            while drains:
                                drains.pop(0)()
                        if mk + 1 < NT:
                            s_cur = s_tile(h, mk + 1)
                        elif h + 1 < NH:
                            s_cur = s_tile(h + 1, 0)
                        emit_qk((3 if half else 2) if mk > 0 else 0)
                        if hp == KC - 1 and half == 0 and 3 <= mk < 3 + KC - 1:
                            # last pair has no q/k filler: pre-accumulate
                            # the first proj tile's psa partials (head
                            # pairs 0-4 are done) in the idle ps_qk bank
                            kp = mk - 3
                            if kp == 0:
                                psa0[0] = ps_qk.tile([P, 512], F32,
                                                     tag="psq2", name="psq2")
                            mm(psa0[0][:], oT[kp][:, 0:P],
                               wproj_sb[kp][:, 0:512],
                               start=(kp == 0), stop=False)
                        for n2 in range(NT2):
                            mm(pv[n2][:], vaug[mk][:, h * VP: h * VP + VA],
                               et[:, n2 * 512:(n2 + 1) * 512],
                               start=(mk == 0), stop=(mk == NT - 1))
                    for n2 in range(NT2):
                        tq = slice(n2 * 512, (n2 + 1) * 512)
                        sums = small.tile([1, 512], F32, tag="rb", name="rb")
                        nc.vector.tensor_copy(sums[:], pv[n2][HD:VA, :])
                        sbc = small.tile([HD, 512], F32, tag="sbc",
                                         name="sbc")
                        nc.gpsimd.partition_broadcast(sbc[:], sums[:])
                        bc = small.tile([HD, 512], F32, tag="bc", name="bc")
                        nc.vector.reciprocal_approx_fast(bc[:], sbc[:])
                        nc.vector.tensor_mul(
                            oT[hp][hr, tq], pv[n2][0:HD, :], bc[:])
                while drains:
                    drains.pop(0)()

            # ---------------- output projection ----------------
            # Lives inside the attention scope, reusing the ps_s PSUM
            # ring ([128,1024] = psa 0:512 + psb 512:768) — no pool-close
            # barrier, so the first tiles' k<5 partials (which only need
            # head pairs 0-4) keep the PE busy while the last pair's
            # normalize drains through DVE/gpsimd.
            def proj_mms(t, ps, ks):
                tq = slice(t * P, (t + 1) * P)
                for k in ks:
                    mm(ps[:, 0:512], oT[k][:, tq], wproj_sb[k][:, 0:512],
                       start=(k == 0), stop=(k == KC - 1))
                    mm(ps[:, 512:768], oT[k][:, tq],
                       wproj_sb[k][:, 512:768],
                       start=(k == 0), stop=(k == KC - 1))

            def proj_drain(t, psa_ap, psb_ap):
                tq = slice(t * P, (t + 1) * P)
                ot = out_pool.tile([P, C], F32, tag="out", name="outt")
                nc.vector.tensor_add(ot[:, 0:512], psa_ap,
                                     bias_bc[:, 0:512])
                nc.sync.dma_start(out_d.ap()[tq, 0:512], ot[:, 0:512])
                nc.vector.tensor_add(ot[:, 512:768], psb_ap,
                                     bias_bc[:, 512:768])
                nc.scalar.dma_start(out_d.ap()[tq, 512:768],
                                    ot[:, 512:768])

            # t=0: psa partials already accumulated in ps_qk during pair
            # 5; add the psb partials, then t=1 partials, then closers.
            ps0 = ps_s.tile([P, N], F32, tag="s", name="s")
            for k in range(KC - 1):
                mm(ps0[:, 512:768], oT[k][:, 0:P], wproj_sb[k][:, 512:768],
                   start=(k == 0), stop=False)
            ps1 = ps_s.tile([P, N], F32, tag="s", name="s")
            proj_mms(1, ps1, range(KC - 1))
            mm(psa0[0][:], oT[KC - 1][:, 0:P], wproj_sb[KC - 1][:, 0:512],
               start=False, stop=True)
            mm(ps0[:, 512:768], oT[KC - 1][:, 0:P],
               wproj_sb[KC - 1][:, 512:768], start=False, stop=True)
            proj_drain(0, psa0[0][:], ps0[:, 512:768])
            proj_mms(1, ps1, [KC - 1])
            proj_drain(1, ps1[:, 0:512], ps1[:, 512:768])
            for t in range(2, NT):
                ps = ps_s.tile([P, N], F32, tag="s", name="s")
                proj_mms(t, ps, range(KC))
                proj_drain(t, ps[:, 0:512], ps[:, 512:768])


def build():
    if "nc" in _CACHE:
        return _CACHE["nc"]
    nc = bacc.Bacc("TRN2", target_bir_lowering=False, debug=False)
    with tile.TileContext(nc) as tc:
        _emit(nc, tc)
    nc.compile()
    _CACHE["nc"] = nc
    return nc


def make_in_maps(x, w_qkv, w_proj, b_proj):
    x = np.asarray(x, dtype=np.float32)
    w_qkv = np.asarray(w_qkv, dtype=np.float32).astype(ml_dtypes.bfloat16)
    w_proj = np.asarray(w_proj, dtype=np.float32).astype(ml_dtypes.bfloat16)
    b_proj = np.ascontiguousarray(
        np.asarray(b_proj, dtype=np.float32).reshape(1, C))
    return [
        {
            "xT": np.ascontiguousarray(x[i].T.astype(ml_dtypes.bfloat16)),
            "w_qkv": w_qkv,
            "w_proj": w_proj,
            "b_proj": b_proj,
        }
        for i in range(B)
    ]


def run(x, w_qkv, w_proj, b_proj, **spmd_kwargs):
    nc = build()
    in_maps = make_in_maps(x, w_qkv, w_proj, b_proj)
    res = run_bass_kernel_spmd(nc, in_maps, core_ids=list(range(B)),
                               **spmd_kwargs)
    out = np.stack([res.results[i]["out"] for i in range(B)])
    return out.astype(np.float32), res


def kernel(x, w_qkv, w_proj, b_proj, H=None, W=None, **_ignored):
    out, _ = run(x, w_qkv, w_proj, b_proj)
    return out


# revision 34
# speedup vs baseline: 1.1579x; 1.1579x over previous
"""Trainium2 Bass kernel for nn_Attention (B=8, N=1024, C=768, 12 heads).

Sharding: pure data-parallel over batch — 8 cores, one batch element per
core, full weights replicated to every core. No collectives.

Per-core dataflow (tokens N=1024, channels C=768, heads 12 x 64), all
tensor-engine matmuls in bf16 (f32 PSUM accumulation):
  stage 0: PE warm-up burst (HAM clock gate) while the input DMAs land;
           bias broadcast via gpsimd partition_broadcast.
  stage 1: v = xT_tile.T @ w_v, token-major, stored as v_aug per-head
           tiles [128, 128]: cols 0:64 = v, col 64 = ones, rest unused.
           Pair-0 q/k projection groups interleaved into the tail.
  stage 2 (attention, one head at a time, head pair hp shares qkT tiles
           in partitions 0-63 / 64-127):
           S^T[tk,tq] = kT_h.T @ qT_h            (K=64)
           attn_exp = exp(scale * S^T)           (ACT, PSUM -> bf16 SBUF)
           O^T_unnorm[65,tq] = v_aug.T @ attn_exp — the ones column makes
           row 64 the softmax denominator for free
           normalize: DVE reciprocal of the sums row, gpsimd
           partition_broadcast to 64 partitions, DVE multiply -> oT bf16.
           The q/k projection of the NEXT head pair (matmuls + DVE
           PSUM->SBUF drains) is interleaved into the mk loop so the PE
           never starves while ACT computes exp; the next head's first
           score tile is prefetched at mk==7.
  stage 3: final[tq,:] = O^T_norm.T @ w_proj; bias added during the
           PSUM->SBUF drain (DVE); per-half DMA out.

Inputs x/w_qkv/w_proj are pre-converted to bf16 on the host (x also
pre-transposed) — halves DMA and runs the PE at 1 cycle/row.
"""

import os
import sys

import numpy as np

for _p in ("/opt/trn_rl_repo",):
    if os.path.isdir(_p) and _p not in sys.path:
        sys.path.append(_p)

import ml_dtypes

import concourse.bacc as bacc
import concourse.mybir as mybir
import concourse.tile as tile
from concourse.bass_utils import run_bass_kernel_spmd

F32 = mybir.dt.float32
BF16 = mybir.dt.bfloat16
EXP = mybir.ActivationFunctionType.Exp

P = 128
B, N, C = 8, 1024, 768
NH, HD = 12, 64
C3 = 3 * C
KC = C // P          # 6 contraction tiles over channels
NT = N // P          # 8 token tiles of 128
NT2 = N // 512       # 2 token slices of 512
VA = HD + 1          # 65: head dim + ones column
VP = VA              # per-head stride inside v_aug tiles (dense, no pad)
SCALE = float(HD) ** -0.5

_CACHE = {}


def _emit(nc, tc):
    xT_d = nc.dram_tensor("xT", [C, N], BF16, kind="ExternalInput")
    wqkv_d = nc.dram_tensor("w_qkv", [C, C3], BF16, kind="ExternalInput")
    wproj_d = nc.dram_tensor("w_proj", [C, C], BF16, kind="ExternalInput")
    bproj_d = nc.dram_tensor("b_proj", [1, C], F32, kind="ExternalInput")
    out_d = nc.dram_tensor("out", [N, C], F32, kind="ExternalOutput")

    mm = nc.tensor.matmul

    from contextlib import ExitStack

    with ExitStack() as es:
        const = es.enter_context(tc.tile_pool(name="const", bufs=1))
        big = es.enter_context(tc.tile_pool(name="big", bufs=1))

        bproj_sb = const.tile([1, C], F32, tag="bproj", name="bproj")
        nc.sync.dma_start(bproj_sb[:], bproj_d.ap())
        bias_bc = const.tile([P, C], F32, tag="bias_bc", name="bias_bc")

        xT = [big.tile([P, N], BF16, tag=f"xT{k}", name=f"xT{k}")
              for k in range(KC)]
        wqv = [big.tile([P, C], BF16, tag=f"wqv{k}", name=f"wqv{k}")
               for k in range(KC)]
        wqk = [big.tile([P, 2 * C], BF16, tag=f"wqk{k}", name=f"wqk{k}")
               for k in range(KC)]
        wpa = big.tile([P, KC * C], BF16, tag="wpa", name="wpa")
        wproj_sb = [wpa[:, k * C:(k + 1) * C] for k in range(KC)]
        qkT = [big.tile([P, N], BF16, tag=f"qk{m}", name=f"qk{m}")
               for m in range(2 * KC)]
        vaug = [big.tile([P, NH * VP], BF16, tag=f"va{t}", name=f"va{t}")
                for t in range(NT)]
        oT = [big.tile([P, N], BF16, tag=f"oT{i}", name=f"oT{i}")
              for i in range(KC)]

        # ---------------- stage 0 + 1: warmup, DMA in, v ----------------
        with tc.tile_pool(name="warmp", bufs=1) as warmp, \
             tc.tile_pool(name="ps_w", bufs=1, space="PSUM") as ps_w, \
             tc.tile_pool(name="ps_v", bufs=2, space="PSUM") as ps_v, \
             tc.tile_pool(name="ps_q0", bufs=2, space="PSUM") as ps_q0:
            # Warm the PE clock (HAM) with dummy matmuls while DMAs land.
            warm_sb = warmp.tile([P, 512], BF16, tag="warm", name="warm")
            nc.vector.memset(warm_sb[:], 0.0)
            warm_ps = ps_w.tile([P, 512], F32, tag="warmps", name="warmps")
            for _ in range(10):
                mm(warm_ps[:], warm_sb[:, 0:P], warm_sb[:],
                   start=True, stop=True)

            # Each issuing engine owns a hardware DMA queue: transfers are
            # FIFO within a queue but share HBM bandwidth across queues
            # (the gpsimd-issued queue gets the biggest share). Enqueue in
            # need-order everywhere: v-projection inputs (xT + v-columns,
            # k-ascending) first on every queue, then q/k weight columns,
            # then w_proj.
            urgent = []
            for k in range(KC):
                urgent.append((xT[k][:], xT_d.ap()[k * P:(k + 1) * P, :]))
                urgent.append((wqv[k][:],
                               wqkv_d.ap()[k * P:(k + 1) * P, 2 * C:]))
            # weighted split: gpsimd queue drains ~3x faster
            engs = [nc.gpsimd, nc.gpsimd, nc.sync, nc.gpsimd, nc.gpsimd,
                    nc.scalar] * 2
            for (dst, src), eng in zip(urgent, engs):
                eng.dma_start(dst, src)
            for k in range(KC):
                eng = (nc.gpsimd, nc.gpsimd, nc.scalar,
                       nc.gpsimd, nc.gpsimd, nc.sync)[k]
                eng.dma_start(wqk[k][:],
                              wqkv_d.ap()[k * P:(k + 1) * P, 0:2 * C])
            wp_src = wproj_d.ap().rearrange("(k p) j -> p k j", p=P)
            nc.gpsimd.dma_start(wpa[:].rearrange("p (k j) -> p k j", j=C),
                                wp_src)
            nc.gpsimd.partition_broadcast(bias_bc[:], bproj_sb[:])

            def qk_group(m, n2, drains):
                """One q/k projection group: 6 matmuls + DVE drain."""
                ps = ps_q0.tile([P, 512], F32, tag="psq", name="psq")
                for k in range(KC):
                    mm(ps[:], wqk[k][:, m * P:(m + 1) * P],
                       xT[k][:, n2 * 512:(n2 + 1) * 512],
                       start=(k == 0), stop=(k == KC - 1))
                drains.append(
                    lambda m=m, n2=n2, ps=ps: nc.vector.tensor_copy(
                        qkT[m][:, n2 * 512:(n2 + 1) * 512], ps[:]))

            # v: token-major [tokens 128, feat], scattered into v_aug tiles
            # (per-head stride VP=65: cols 0:64 data, col 64 ones).
            for t in range(NT):
                nc.gpsimd.memset(
                    vaug[t][:].rearrange("p (h m) -> p h m", m=VP)
                    [:, :, HD:HD + 1], 1.0)
                psa = ps_v.tile([P, 512], F32, tag="psva", name="psva")
                psb = ps_v.tile([P, 256], F32, tag="psvb", name="psvb")
                for k in range(KC):
                    mm(psa[:], xT[k][:, t * P:(t + 1) * P],
                       wqv[k][:, 0:512],
                       start=(k == 0), stop=(k == KC - 1))
                    mm(psb[:], xT[k][:, t * P:(t + 1) * P],
                       wqv[k][:, 512:768],
                       start=(k == 0), stop=(k == KC - 1))
                dst = vaug[t][:].rearrange("p (h m) -> p h m", m=VP)
                srca = psa[:].rearrange("p (h m) -> p h m", m=HD)
                srcb = psb[:].rearrange("p (h m) -> p h m", m=HD)
                nc.vector.tensor_copy(dst[:, 0:8, 0:HD], srca)
                nc.vector.tensor_copy(dst[:, 8:12, 0:HD], srcb)
            # pair-0 q/k projection (q/k weight columns have landed by now)
            d0 = []
            for m in (0, KC):
                for n2 in range(NT2):
                    qk_group(m, n2, d0)
                    d0.pop(0)()

        # ---------------- stage 2: attention ----------------
        with tc.tile_pool(name="attn", bufs=6) as attn_pool, \
             tc.tile_pool(name="small", bufs=4) as small, \
             tc.tile_pool(name="outp", bufs=3) as out_pool, \
             tc.tile_pool(name="ps_s", bufs=2, space="PSUM") as ps_s, \
             tc.tile_pool(name="ps_pv", bufs=3, space="PSUM") as ps_pv, \
             tc.tile_pool(name="ps_qk", bufs=1, space="PSUM") as ps_qk:

            def s_tile(h, mk):
                """Score tile S^T[tk block mk, all tq] for head h."""
                qt, kt = qkT[h // 2], qkT[KC + h // 2]
                hr = slice((h % 2) * HD, (h % 2) * HD + HD)
                s = ps_s.tile([P, N], F32, tag="s", name="s")
                tkc = slice(mk * P, (mk + 1) * P)
                for n2 in range(NT2):
                    mm(s[:, n2 * 512:(n2 + 1) * 512], kt[hr, tkc],
                       qt[hr, n2 * 512:(n2 + 1) * 512],
                       start=True, stop=True)
                return s

            # pending q/k projection work for the next pair, interleaved
            # one matmul at a time into the attention inner loop
            pend = []    # flat list of (m, n2, k) matmuls still to emit
            drains = []  # deferred DVE PSUM->SBUF drains
            cur_ps = [None]
            psa0 = [None]  # proj t=0 psa partials, filled during pair 5

            def emit_qk(n_items):
                for _ in range(n_items):
                    if not pend:
                        return
                    m, n2, k = pend.pop(0)
                    if k == 0:
                        while drains:
                            drains.pop(0)()
                        cur_ps[0] = ps_qk.tile([P, 512], F32, tag="psq2",
                                               name="psq2")
                    ps = cur_ps[0]
                    mm(ps[:], wqk[k][:, m * P:(m + 1) * P],
                       xT[k][:, n2 * 512:(n2 + 1) * 512],
                       start=(k == 0), stop=(k == KC - 1))
                    if k == KC - 1:
                        drains.append(
                            lambda m=m, n2=n2, ps=ps: nc.vector.tensor_copy(
                                qkT[m][:, n2 * 512:(n2 + 1) * 512], ps[:]))

            s_cur = s_tile(0, 0)
            for hp in range(KC):
                if hp + 1 < KC:
                    pend = [(m, n2, k)
                            for m in (hp + 1, KC + hp + 1)
                            for n2 in range(NT2)
                            for k in range(KC)]
                for half in range(2):
                    h = 2 * hp + half
                    hr = slice(half * HD, (half + 1) * HD)
                    pv = [ps_pv.tile([VA, 512], F32, tag="pv", name="pv")
                          for _ in range(NT2)]
                    for mk in range(NT):
                        et = attn_pool.tile([P, N], BF16, tag="e", name="e")
                        nc.scalar.activation(et[:], s_cur[:], EXP,
                                             scale=SCALE)
                        if half == 1 and mk == 5:
                            # finish the next pair's q/k projection well
                            # before the cross-pair score prefetch reads it
                            emit_qk(len(pend))
                            while drains:
                                drains.pop(0)()
                        if mk + 1 < NT:
                            s_cur = s_tile(h, mk + 1)
                        elif h + 1 < NH:
                            s_cur = s_tile(h + 1, 0)
                        emit_qk((3 if half else 2) if mk > 0 else 0)
                        for n2 in range(NT2):
                            mm(pv[n2][:], vaug[mk][:, h * VP: h * VP + VA],
                               et[:, n2 * 512:(n2 + 1) * 512],
                               start=(mk == 0), stop=(mk == NT - 1))
                    for n2 in range(NT2):
                        tq = slice(n2 * 512, (n2 + 1) * 512)
                        sums = small.tile([1, 512], F32, tag="rb", name="rb")
                        nc.vector.tensor_copy(sums[:], pv[n2][HD:VA, :])
                        sbc = small.tile([HD, 512], F32, tag="sbc",
                                         name="sbc")
                        nc.gpsimd.partition_broadcast(sbc[:], sums[:])
                        bc = small.tile([HD, 512], F32, tag="bc", name="bc")
                        nc.vector.reciprocal_approx_fast(bc[:], sbc[:])
                        nc.vector.tensor_mul(
                            oT[hp][hr, tq], pv[n2][0:HD, :], bc[:])
                while drains:
                    drains.pop(0)()

            # ---------------- output projection ----------------
            # Lives inside the attention scope, reusing the ps_s PSUM
            # ring ([128,1024] = psa 0:512 + psb 512:768) — no pool-close
            # barrier, so the first tiles' k<5 partials (which only need
            # head pairs 0-4) keep the PE busy while the last pair's
            # normalize drains through DVE/gpsimd.
            def proj_mms(t, ps, ks):
                tq = slice(t * P, (t + 1) * P)
                for k in ks:
                    mm(ps[:, 0:512], oT[k][:, tq], wproj_sb[k][:, 0:512],
                       start=(k == 0), stop=(k == KC - 1))
                    mm(ps[:, 512:768], oT[k][:, tq],
                       wproj_sb[k][:, 512:768],
                       start=(k == 0), stop=(k == KC - 1))

            def proj_drain(t, psa_ap, psb_ap):
                tq = slice(t * P, (t + 1) * P)
                ot = out_pool.tile([P, C], F32, tag="out", name="outt")
                nc.vector.tensor_add(ot[:, 0:512], psa_ap,
                                     bias_bc[:, 0:512])
                nc.sync.dma_start(out_d.ap()[tq, 0:512], ot[:, 0:512])
                nc.vector.tensor_add(ot[:, 512:768], psb_ap,
                                     bias_bc[:, 512:768])
                nc.scalar.dma_start(out_d.ap()[tq, 512:768],
                                    ot[:, 512:768])

            held = []
            for t in range(2):
                ps = ps_s.tile([P, N], F32, tag="s", name="s")
                proj_mms(t, ps, range(KC - 1))
                held.append((t, ps))
            for t, ps in held:
                proj_mms(t, ps, [KC - 1])
                proj_drain(t, ps[:, 0:512], ps[:, 512:768])
            for t in range(2, NT):
                ps = ps_s.tile([P, N], F32, tag="s", name="s")
                proj_mms(t, ps, range(KC))
                proj_drain(t, ps[:, 0:512], ps[:, 512:768])


def build():
    if "nc" in _CACHE:
        return _CACHE["nc"]
    nc = bacc.Bacc("TRN2", target_bir_lowering=False, debug=False)
    with tile.TileContext(nc) as tc:
        _emit(nc, tc)
    nc.compile()
    _CACHE["nc"] = nc
    return nc


def make_in_maps(x, w_qkv, w_proj, b_proj):
    x = np.asarray(x, dtype=np.float32)
    w_qkv = np.asarray(w_qkv, dtype=np.float32).astype(ml_dtypes.bfloat16)
    w_proj = np.asarray(w_proj, dtype=np.float32).astype(ml_dtypes.bfloat16)
    b_proj = np.ascontiguousarray(
        np.asarray(b_proj, dtype=np.float32).reshape(1, C))
    return [
        {
            "xT": np.ascontiguousarray(x[i].T.astype(ml_dtypes.bfloat16)),
            "w_qkv": w_qkv,
            "w_proj": w_proj,
            "b_proj": b_proj,
        }
        for i in range(B)
    ]


def run(x, w_qkv, w_proj, b_proj, **spmd_kwargs):
    nc = build()
    in_maps = make_in_maps(x, w_qkv, w_proj, b_proj)
    res = run_bass_kernel_spmd(nc, in_maps, core_ids=list(range(B)),
                               **spmd_kwargs)
    out = np.stack([res.results[i]["out"] for i in range(B)])
    return out.astype(np.float32), res


def kernel(x, w_qkv, w_proj, b_proj, H=None, W=None, **_ignored):
    out, _ = run(x, w_qkv, w_proj, b_proj)
    return out


# revision 36
# speedup vs baseline: 1.1873x; 1.0254x over previous
"""Trainium2 Bass kernel for nn_Attention (B=8, N=1024, C=768, 12 heads).

Sharding: pure data-parallel over batch — 8 cores, one batch element per
core, full weights replicated to every core. No collectives.

Per-core dataflow (tokens N=1024, channels C=768, heads 12 x 64), all
tensor-engine matmuls in bf16 (f32 PSUM accumulation):
  stage 0: PE warm-up burst (HAM clock gate) while the input DMAs land;
           bias broadcast via gpsimd partition_broadcast.
  stage 1: v = xT_tile.T @ w_v, token-major, stored as v_aug per-head
           tiles [128, 128]: cols 0:64 = v, col 64 = ones, rest unused.
           Pair-0 q/k projection groups interleaved into the tail.
  stage 2 (attention, one head at a time, head pair hp shares qkT tiles
           in partitions 0-63 / 64-127):
           S^T[tk,tq] = kT_h.T @ qT_h            (K=64)
           attn_exp = exp(scale * S^T)           (ACT, PSUM -> bf16 SBUF)
           O^T_unnorm[65,tq] = v_aug.T @ attn_exp — the ones column makes
           row 64 the softmax denominator for free
           normalize: DVE reciprocal of the sums row, gpsimd
           partition_broadcast to 64 partitions, DVE multiply -> oT bf16.
           The q/k projection of the NEXT head pair (matmuls + DVE
           PSUM->SBUF drains) is interleaved into the mk loop so the PE
           never starves while ACT computes exp; the next head's first
           score tile is prefetched at mk==7.
  stage 3: final[tq,:] = O^T_norm.T @ w_proj; bias added during the
           PSUM->SBUF drain (DVE); per-half DMA out.

Inputs x/w_qkv/w_proj are pre-converted to bf16 on the host (x also
pre-transposed) — halves DMA and runs the PE at 1 cycle/row.
"""

import os
import sys

import numpy as np

for _p in ("/opt/trn_rl_repo",):
    if os.path.isdir(_p) and _p not in sys.path:
        sys.path.append(_p)

import ml_dtypes

import concourse.bacc as bacc
import concourse.mybir as mybir
import concourse.tile as tile
from concourse.bass_utils import run_bass_kernel_spmd

F32 = mybir.dt.float32
BF16 = mybir.dt.bfloat16
EXP = mybir.ActivationFunctionType.Exp

P = 128
B, N, C = 8, 1024, 768
NH, HD = 12, 64
C3 = 3 * C
KC = C // P          # 6 contraction tiles over channels
NT = N // P          # 8 token tiles of 128
NT2 = N // 512       # 2 token slices of 512
VA = HD + 1          # 65: head dim + ones column
VP = VA              # per-head stride inside v_aug tiles (dense, no pad)
SCALE = float(HD) ** -0.5

_CACHE = {}


def _emit(nc, tc):
    xT_d = nc.dram_tensor("xT", [C, N], BF16, kind="ExternalInput")
    wqkv_d = nc.dram_tensor("w_qkv", [C, C3], BF16, kind="ExternalInput")
    wproj_d = nc.dram_tensor("w_proj", [C, C], BF16, kind="ExternalInput")
    bproj_d = nc.dram_tensor("b_proj", [1, C], F32, kind="ExternalInput")
    out_d = nc.dram_tensor("out", [N, C], F32, kind="ExternalOutput")

    mm = nc.tensor.matmul

    from contextlib import ExitStack

    with ExitStack() as es:
        const = es.enter_context(tc.tile_pool(name="const", bufs=1))
        big = es.enter_context(tc.tile_pool(name="big", bufs=1))

        bproj_sb = const.tile([1, C], F32, tag="bproj", name="bproj")
        nc.sync.dma_start(bproj_sb[:], bproj_d.ap())
        bias_bc = const.tile([P, C], F32, tag="bias_bc", name="bias_bc")

        xT = [big.tile([P, N], BF16, tag=f"xT{k}", name=f"xT{k}")
              for k in range(KC)]
        wqv = [big.tile([P, C], BF16, tag=f"wqv{k}", name=f"wqv{k}")
               for k in range(KC)]
        wqk = [big.tile([P, 2 * C], BF16, tag=f"wqk{k}", name=f"wqk{k}")
               for k in range(KC)]
        wpa = big.tile([P, KC * C], BF16, tag="wpa", name="wpa")
        wproj_sb = [wpa[:, k * C:(k + 1) * C] for k in range(KC)]
        qkT = [big.tile([P, N], BF16, tag=f"qk{m}", name=f"qk{m}")
               for m in range(2 * KC)]
        vaug = [big.tile([P, NH * VP], BF16, tag=f"va{t}", name=f"va{t}")
                for t in range(NT)]
        oT = [big.tile([P, N], BF16, tag=f"oT{i}", name=f"oT{i}")
              for i in range(KC)]

        # ---------------- stage 0 + 1: warmup, DMA in, v ----------------
        with tc.tile_pool(name="warmp", bufs=1) as warmp, \
             tc.tile_pool(name="ps_w", bufs=2, space="PSUM") as ps_w, \
             tc.tile_pool(name="ps_v", bufs=2, space="PSUM") as ps_v, \
             tc.tile_pool(name="ps_q0", bufs=2, space="PSUM") as ps_q0:
            # Warm the PE clock (HAM) with dummy matmuls while DMAs land;
            # two PSUM buffers so the WAW dependency doesn't gap them.
            warm_sb = warmp.tile([P, 512], BF16, tag="warm", name="warm")
            nc.vector.memset(warm_sb[:], 0.0)
            for _ in range(12):
                warm_ps = ps_w.tile([P, 512], F32, tag="warmps",
                                    name="warmps")
                mm(warm_ps[:], warm_sb[:, 0:P], warm_sb[:],
                   start=True, stop=True)

            # Each issuing engine owns a hardware DMA queue: transfers are
            # FIFO within a queue but share HBM bandwidth across queues
            # (the gpsimd-issued queue gets the biggest share). Enqueue in
            # need-order everywhere: v-projection inputs (xT + v-columns,
            # k-ascending) first on every queue, then q/k weight columns,
            # then w_proj.
            urgent = []
            for k in range(KC):
                urgent.append((xT[k][:], xT_d.ap()[k * P:(k + 1) * P, :]))
                urgent.append((wqv[k][:],
                               wqkv_d.ap()[k * P:(k + 1) * P, 2 * C:]))
            # weighted split: gpsimd queue drains ~3x faster
            engs = [nc.gpsimd, nc.gpsimd, nc.sync, nc.gpsimd, nc.gpsimd,
                    nc.scalar] * 2
            for (dst, src), eng in zip(urgent, engs):
                eng.dma_start(dst, src)
            for k in range(KC):
                eng = (nc.gpsimd, nc.gpsimd, nc.scalar,
                       nc.gpsimd, nc.gpsimd, nc.sync)[k]
                eng.dma_start(wqk[k][:],
                              wqkv_d.ap()[k * P:(k + 1) * P, 0:2 * C])
            wp_src = wproj_d.ap().rearrange("(k p) j -> p k j", p=P)
            nc.gpsimd.dma_start(wpa[:].rearrange("p (k j) -> p k j", j=C),
                                wp_src)
            nc.gpsimd.partition_broadcast(bias_bc[:], bproj_sb[:])

            def qk_group(m, n2, drains):
                """One q/k projection group: 6 matmuls + DVE drain."""
                ps = ps_q0.tile([P, 512], F32, tag="psq", name="psq")
                for k in range(KC):
                    mm(ps[:], wqk[k][:, m * P:(m + 1) * P],
                       xT[k][:, n2 * 512:(n2 + 1) * 512],
                       start=(k == 0), stop=(k == KC - 1))
                drains.append(
                    lambda m=m, n2=n2, ps=ps: nc.vector.tensor_copy(
                        qkT[m][:, n2 * 512:(n2 + 1) * 512], ps[:]))

            # v: token-major [tokens 128, feat], scattered into v_aug tiles
            # (per-head stride VP=65: cols 0:64 data, col 64 ones).
            for t in range(NT):
                nc.gpsimd.memset(
                    vaug[t][:].rearrange("p (h m) -> p h m", m=VP)
                    [:, :, HD:HD + 1], 1.0)
                psa = ps_v.tile([P, 512], F32, tag="psva", name="psva")
                psb = ps_v.tile([P, 256], F32, tag="psvb", name="psvb")
                for k in range(KC):
                    mm(psa[:], xT[k][:, t * P:(t + 1) * P],
                       wqv[k][:, 0:512],
                       start=(k == 0), stop=(k == KC - 1))
                    mm(psb[:], xT[k][:, t * P:(t + 1) * P],
                       wqv[k][:, 512:768],
                       start=(k == 0), stop=(k == KC - 1))
                dst = vaug[t][:].rearrange("p (h m) -> p h m", m=VP)
                srca = psa[:].rearrange("p (h m) -> p h m", m=HD)
                srcb = psb[:].rearrange("p (h m) -> p h m", m=HD)
                nc.vector.tensor_copy(dst[:, 0:8, 0:HD], srca)
                nc.vector.tensor_copy(dst[:, 8:12, 0:HD], srcb)
            # pair-0 q/k projection (q/k weight columns have landed by now)
            d0 = []
            for m in (0, KC):
                for n2 in range(NT2):
                    qk_group(m, n2, d0)
                    d0.pop(0)()

        # ---------------- stage 2: attention ----------------
        with tc.tile_pool(name="attn", bufs=6) as attn_pool, \
             tc.tile_pool(name="small", bufs=4) as small, \
             tc.tile_pool(name="outp", bufs=3) as out_pool, \
             tc.tile_pool(name="ps_s", bufs=2, space="PSUM") as ps_s, \
             tc.tile_pool(name="ps_pv", bufs=3, space="PSUM") as ps_pv, \
             tc.tile_pool(name="ps_qk", bufs=1, space="PSUM") as ps_qk:

            def s_tile(h, mk):
                """Score tile S^T[tk block mk, all tq] for head h."""
                qt, kt = qkT[h // 2], qkT[KC + h // 2]
                hr = slice((h % 2) * HD, (h % 2) * HD + HD)
                s = ps_s.tile([P, N], F32, tag="s", name="s")
                tkc = slice(mk * P, (mk + 1) * P)
                for n2 in range(NT2):
                    mm(s[:, n2 * 512:(n2 + 1) * 512], kt[hr, tkc],
                       qt[hr, n2 * 512:(n2 + 1) * 512],
                       start=True, stop=True)
                return s

            # pending q/k projection work for the next pair, interleaved
            # one matmul at a time into the attention inner loop
            pend = []    # flat list of (m, n2, k) matmuls still to emit
            drains = []  # deferred DVE PSUM->SBUF drains
            cur_ps = [None]
            psa0 = [None]  # proj t=0 psa partials, filled during pair 5

            def emit_qk(n_items):
                for _ in range(n_items):
                    if not pend:
                        return
                    m, n2, k = pend.pop(0)
                    if k == 0:
                        while drains:
                            drains.pop(0)()
                        cur_ps[0] = ps_qk.tile([P, 512], F32, tag="psq2",
                                               name="psq2")
                    ps = cur_ps[0]
                    mm(ps[:], wqk[k][:, m * P:(m + 1) * P],
                       xT[k][:, n2 * 512:(n2 + 1) * 512],
                       start=(k == 0), stop=(k == KC - 1))
                    if k == KC - 1:
                        drains.append(
                            lambda m=m, n2=n2, ps=ps: nc.vector.tensor_copy(
                                qkT[m][:, n2 * 512:(n2 + 1) * 512], ps[:]))

            s_cur = s_tile(0, 0)
            for hp in range(KC):
                if hp + 1 < KC:
                    pend = [(m, n2, k)
                            for m in (hp + 1, KC + hp + 1)
                            for n2 in range(NT2)
                            for k in range(KC)]
                for half in range(2):
                    h = 2 * hp + half
                    hr = slice(half * HD, (half + 1) * HD)
                    pv = [ps_pv.tile([VA, 512], F32, tag="pv", name="pv")
                          for _ in range(NT2)]
                    for mk in range(NT):
                        et = attn_pool.tile([P, N], BF16, tag="e", name="e")
                        nc.scalar.activation(et[:], s_cur[:], EXP,
                                             scale=SCALE)
                        if half == 1 and mk == 5:
                            # finish the next pair's q/k projection well
                            # before the cross-pair score prefetch reads it
                            emit_qk(len(pend))
                            while drains:
                                drains.pop(0)()
                        if mk + 1 < NT:
                            s_cur = s_tile(h, mk + 1)
                        elif h + 1 < NH:
                            s_cur = s_tile(h + 1, 0)
                        emit_qk((3 if half else 2) if mk > 0 else 0)
                        for n2 in range(NT2):
                            mm(pv[n2][:], vaug[mk][:, h * VP: h * VP + VA],
                               et[:, n2 * 512:(n2 + 1) * 512],
                               start=(mk == 0), stop=(mk == NT - 1))
                    for n2 in range(NT2):
                        tq = slice(n2 * 512, (n2 + 1) * 512)
                        sums = small.tile([1, 512], F32, tag="rb", name="rb")
                        nc.vector.tensor_copy(sums[:], pv[n2][HD:VA, :])
                        sbc = small.tile([HD, 512], F32, tag="sbc",
                                         name="sbc")
                        nc.gpsimd.partition_broadcast(sbc[:], sums[:])
                        bc = small.tile([HD, 512], F32, tag="bc", name="bc")
                        nc.vector.reciprocal_approx_fast(bc[:], sbc[:])
                        nc.vector.tensor_mul(
                            oT[hp][hr, tq], pv[n2][0:HD, :], bc[:])
                while drains:
                    drains.pop(0)()

            # ---------------- output projection ----------------
            # Lives inside the attention scope, reusing the ps_s PSUM
            # ring ([128,1024] = psa 0:512 + psb 512:768) — no pool-close
            # barrier, so the first tiles' k<5 partials (which only need
            # head pairs 0-4) keep the PE busy while the last pair's
            # normalize drains through DVE/gpsimd.
            def proj_mms(t, ps, ks):
                tq = slice(t * P, (t + 1) * P)
                for k in ks:
                    mm(ps[:, 0:512], oT[k][:, tq], wproj_sb[k][:, 0:512],
                       start=(k == 0), stop=(k == KC - 1))
                    mm(ps[:, 512:768], oT[k][:, tq],
                       wproj_sb[k][:, 512:768],
                       start=(k == 0), stop=(k == KC - 1))

            def proj_drain(t, psa_ap, psb_ap):
                tq = slice(t * P, (t + 1) * P)
                ot = out_pool.tile([P, C], F32, tag="out", name="outt")
                nc.vector.tensor_add(ot[:, 0:512], psa_ap,
                                     bias_bc[:, 0:512])
                nc.sync.dma_start(out_d.ap()[tq, 0:512], ot[:, 0:512])
                nc.vector.tensor_add(ot[:, 512:768], psb_ap,
                                     bias_bc[:, 512:768])
                nc.scalar.dma_start(out_d.ap()[tq, 512:768],
                                    ot[:, 512:768])

            held = []
            for t in range(2):
                ps = ps_s.tile([P, N], F32, tag="s", name="s")
                proj_mms(t, ps, range(KC - 1))
                held.append((t, ps))
            for t, ps in held:
                proj_mms(t, ps, [KC - 1])
                proj_drain(t, ps[:, 0:512], ps[:, 512:768])
            for t in range(2, NT):
                ps = ps_s.tile([P, N], F32, tag="s", name="s")
                proj_mms(t, ps, range(KC))
                proj_drain(t, ps[:, 0:512], ps[:, 512:768])


def build():
    if "nc" in _CACHE:
        return _CACHE["nc"]
    nc = bacc.Bacc("TRN2", target_bir_lowering=False, debug=False)
    with tile.TileContext(nc) as tc:
        _emit(nc, tc)
    nc.compile()
    _CACHE["nc"] = nc
    return nc


def make_in_maps(x, w_qkv, w_proj, b_proj):
    x = np.asarray(x, dtype=np.float32)
    w_qkv = np.asarray(w_qkv, dtype=np.float32).astype(ml_dtypes.bfloat16)
    w_proj = np.asarray(w_proj, dtype=np.float32).astype(ml_dtypes.bfloat16)
    b_proj = np.ascontiguousarray(
        np.asarray(b_proj, dtype=np.float32).reshape(1, C))
    return [
        {
            "xT": np.ascontiguousarray(x[i].T.astype(ml_dtypes.bfloat16)),
            "w_qkv": w_qkv,
            "w_proj": w_proj,
            "b_proj": b_proj,
        }
        for i in range(B)
    ]


def run(x, w_qkv, w_proj, b_proj, **spmd_kwargs):
    nc = build()
    in_maps = make_in_maps(x, w_qkv, w_proj, b_proj)
    res = run_bass_kernel_spmd(nc, in_maps, core_ids=list(range(B)),
                               **spmd_kwargs)
    out = np.stack([res.results[i]["out"] for i in range(B)])
    return out.astype(np.float32), res


def kernel(x, w_qkv, w_proj, b_proj, H=None, W=None, **_ignored):
    out, _ = run(x, w_qkv, w_proj, b_proj)
    return out
